# revision 1
# baseline (speedup 1.0000x reference)
"""Trainium2 Bass kernel for nn_Discriminator (RGCN + gated pooling GNN).

v2 strategy (8 NeuronCores, SPMD) — overlap everything with the a-load:
- Shard nodes N=4096 into 8 row-blocks of 512. Each core streams its
  a-shard [4, 512, 4096] fp32 from HBM once (SWDGE fp16 cast) in
  node-BLOCK-major order (128-row blocks outer, relation inner), PE-
  transposing into an SBUF-resident aT (16.8MB fp16).
- After each 128-row block: layer-0 message B0 = a@x0 completes for
  those rows -> h0 chunk -> chunked AllGather (4 small AGs), all hidden
  under the remaining load. Layer-1 accumulation (msg1 += hrel1^T @ aT)
  runs in waves as h0 chunks arrive, also under the load.
- Layer 1 uses the h_rel-first factoring: hrel1[r] = h0 @ w1r computed
  natural-layout on the PE from the AllGathered h0^T chunks (no
  naturalize transpose DMA), accumulated straight into one msg1 PSUM.
- Pooling: per-core segment-sum partial [G, D] -> fp16 ReduceScatter
  (each core owns 64 graphs) -> final MLP on the slice; the host
  concatenates the 8 output slices. No AllReduce, no full final MLP.
- A tiny dummy AllGather at t~0 absorbs cc-stream startup + core skew.
"""
import numpy as np

import concourse.bass as bass
import concourse.bacc as bacc
import concourse.tile as tile
import concourse.mybir as mybir
import concourse.bass_utils as bass_utils

P = 8          # cores
T = 5          # atom types
R = 4          # relations
N = 4096       # nodes
G = 512        # graphs
D = 128        # hidden
NP = N // P    # nodes per core (512)
MT = N // 128  # m-tiles (32)
NB = NP // 128  # row blocks per core (4)
GC = G // P    # graphs per core (64)
F16 = mybir.dt.float16
F32 = mybir.dt.float32
AF = mybir.ActivationFunctionType

_CACHE = {}
DEBUG_TAPS = False

# round rho -> (block, after-chunk-r) emission slot for hrel prep
PREP_SLOT = {0: (2, 0), 1: (3, 0), 2: (3, 2)}  # rho 3 preps in the tail


def _build():
    nc = bacc.Bacc("TRN2", target_bir_lowering=False, debug=False,
                   num_devices=P)

    a_sh = nc.dram_tensor("a_sh", [R, NP, N], F32, kind="ExternalInput")
    # x0 natural layout tiles, hi/lo fp16 planes packed on the last axis
    x0nat = nc.dram_tensor("x0nat", [128, MT, 2 * T], F16,
                           kind="ExternalInput")
    x0To = nc.dram_tensor("x0To", [T, NP], F32, kind="ExternalInput")
    Sm = nc.dram_tensor("Sm", [NP, G], F16, kind="ExternalInput")
    # w0r stacked twice so one fp32 matmul applies hi+lo B0 parts
    w0r = nc.dram_tensor("w0r", [R, 2 * T, D], F32, kind="ExternalInput")
    w1r = nc.dram_tensor("w1r", [R, D, D], F16, kind="ExternalInput")
    w0s = nc.dram_tensor("w0s", [T, D], F32, kind="ExternalInput")
    w1s = nc.dram_tensor("w1s", [D, D], F16, kind="ExternalInput")
    iw1a = nc.dram_tensor("iw1a", [T, D], F32, kind="ExternalInput")
    iw1b = nc.dram_tensor("iw1b", [D, D], F16, kind="ExternalInput")
    iw2 = nc.dram_tensor("iw2", [D, D], F16, kind="ExternalInput")
    jw1a = nc.dram_tensor("jw1a", [T, D], F32, kind="ExternalInput")
    jw1b = nc.dram_tensor("jw1b", [D, D], F16, kind="ExternalInput")
    jw2 = nc.dram_tensor("jw2", [D, D], F16, kind="ExternalInput")
    fw1 = nc.dram_tensor("fw1", [D, D], F16, kind="ExternalInput")
    fw2 = nc.dram_tensor("fw2", [D, 1], F16, kind="ExternalInput")
    # bias columns: 0=b0 1=b1 2=ib1 3=ib2 4=jb1 5=jb2 6=fb1
    bias8 = nc.dram_tensor("bias8", [D, 8], F32, kind="ExternalInput")
    fb2v = nc.dram_tensor("fb2v", [1, 1], F32, kind="ExternalInput")
    ident = nc.dram_tensor("ident", [128, 128], F16, kind="ExternalInput")

    outT = nc.dram_tensor("outT", [1, GC], F32, kind="ExternalOutput")
    if DEBUG_TAPS:
        dbg_h0 = nc.dram_tensor("dbg_h0", [NB, D, P * 128], F16,
                                kind="ExternalOutput")
        dbg_h1 = nc.dram_tensor("dbg_h1", [D, NP], F16,
                                kind="ExternalOutput")
        dbg_pool = nc.dram_tensor("dbg_pool", [D, G], F16,
                                  kind="ExternalOutput")
        dbg_rs = nc.dram_tensor("dbg_rs", [GC, D], F16,
                                kind="ExternalOutput")
        dbg_w1r = nc.dram_tensor("dbg_w1r", [D, R, D], F16,
                                 kind="ExternalOutput")
        dbg_hrel = nc.dram_tensor("dbg_hrel", [128, MT, R, D], F16,
                                  kind="ExternalOutput")

    with tile.TileContext(nc) as tc:
        with (
            tc.tile_pool(name="const", bufs=1) as cp,
            tc.tile_pool(name="ares", bufs=1) as ap_,
            tc.tile_pool(name="hrel", bufs=1) as hp,
            tc.tile_pool(name="h0sp", bufs=1) as h0p,
            tc.tile_pool(name="psM1", bufs=1, space="PSUM") as psM1p,
            tc.tile_pool(name="psHr", bufs=2, space="PSUM") as psHrp,
            tc.tile_pool(name="dram", bufs=1, space="DRAM") as dp,
        ):
            # ---- constants (sync HWDGE queue; SWDGE reserved for a) ----
            ident_sb = cp.tile([128, 128], F16)
            nc.sync.dma_start(ident_sb[:], ident.ap())
            x0n_sb = cp.tile([128, MT, 2 * T], F16)
            nc.sync.dma_start(x0n_sb[:], x0nat.ap())
            x0To_sb = cp.tile([T, NP], F32)
            nc.sync.dma_start(x0To_sb[:], x0To.ap())
            w0r_sb = cp.tile([2 * T, R, D], F32)
            nc.sync.dma_start(w0r_sb[:], w0r.ap().rearrange("r t d -> t r d"))
            w0s_sb = cp.tile([T, D], F32)
            nc.sync.dma_start(w0s_sb[:], w0s.ap())
            bias_sb = cp.tile([D, 8], F32)
            nc.sync.dma_start(bias_sb[:], bias8.ap())
            fb2_sb = cp.tile([1, 1], F32)
            nc.sync.dma_start(fb2_sb[:], fb2v.ap())
            w1r_sb = cp.tile([D, R, D], F16)
            nc.sync.dma_start(w1r_sb[:], w1r.ap().rearrange("r t d -> t r d"))
            w1s_sb = cp.tile([D, D], F16)
            nc.sync.dma_start(w1s_sb[:], w1s.ap())
            # late-phase consts on the scalar HWDGE queue
            S_sb = cp.tile([128, NB, G], F16)
            nc.scalar.dma_start(
                S_sb[:], Sm.ap().rearrange("(a p) g -> p a g", p=128))
            iw1a_sb = cp.tile([T, D], F32)
            nc.scalar.dma_start(iw1a_sb[:], iw1a.ap())
            iw1b_sb = cp.tile([D, D], F16)
            nc.scalar.dma_start(iw1b_sb[:], iw1b.ap())
            iw2_sb = cp.tile([D, D], F16)
            nc.scalar.dma_start(iw2_sb[:], iw2.ap())
            jw1a_sb = cp.tile([T, D], F32)
            nc.scalar.dma_start(jw1a_sb[:], jw1a.ap())
            jw1b_sb = cp.tile([D, D], F16)
            nc.scalar.dma_start(jw1b_sb[:], jw1b.ap())
            jw2_sb = cp.tile([D, D], F16)
            nc.scalar.dma_start(jw2_sb[:], jw2.ap())
            fw1_sb = cp.tile([D, D], F16)
            nc.scalar.dma_start(fw1_sb[:], fw1.ap())
            fw2_sb = cp.tile([D, 1], F16)
            nc.scalar.dma_start(fw2_sb[:], fw2.ap())

            def bias(k):
                return bias_sb[:, k:k + 1]

            # all-zeros row: opens one accumulation group per PSUM bank
            # (PE writes 0*0 with start=True across the full region; every
            # later matmul accumulates with start=False — a PSUM bank may
            # only ever hold one open group at a time)
            zres = cp.tile([1, NP], F16)
            nc.vector.memset(zres[:], 0.0)

            def open_group(ps_ap, width):
                nc.tensor.matmul(ps_ap, zres[0:1, 0:ps_ap.partition_size()],
                                 zres[0:1, 0:width], start=True, stop=False)

            # ---- persistent big tiles ----
            aT = [ap_.tile([128, MT, NP], F16, name=f"aT{r}")
                  for r in range(R)]
            hrel_sb = hp.tile([128, MT, R, D], F16, name="hrel")
            psM1 = psM1p.tile([D, NP], F32, name="psM1")
            h0ch = [h0p.tile([D, 128], F16, name=f"h0ch{b}")
                    for b in range(NB)]
            h0Tb = [h0p.tile([D, P, 128], F16, name=f"h0Tb{b}")
                    for b in range(NB)]

            # ---- collective DRAM buffers ----
            warm_in = dp.tile([1, 8], F16, name="warm_in")
            warm_out = dp.tile([P, 1, 8], F16, addr_space="Shared", name="warm_out")
            ag_in = [dp.tile([D, 128], F16, name=f"ag_in{b}")
                     for b in range(NB)]
            ag_out = [dp.tile([P, D, 128], F16, addr_space="Shared",
                               name=f"ag_out{b}") for b in range(NB)]
            rs_in = dp.tile([G, D], F16, name="rs_in")
            rs_out = dp.tile([GC, D], F16, name="rs_out")

            # warm the cc stream / absorb core-launch skew early; the
            # trigger itself is deferred (pending_cc) so the gpsimd queue
            # issues a-load descriptors first and never starves the load.
            nc.sync.dma_start(warm_in[:], ident_sb[0:1, 0:8])
            pending_cc = [lambda: nc.gpsimd.collective_compute(
                "AllGather", mybir.AluOpType.bypass,
                replica_groups=[list(range(P))],
                ins=[warm_in[:]], outs=[warm_out[:]])]

            # ---- B1 (layer-1 message) emission bookkeeping ----
            m1_count = [0] * NB      # ops emitted per psM1 column group
            chunk_done = set()       # (b, r) a-chunks transposed
            prepped = set()          # rounds with hrel1 ready
            emitted = set()          # (r, mt, c) singles emitted

            def emit_b1_single(r, mt, c):
                nc.tensor.matmul(
                    psM1[:, c * 128:(c + 1) * 128],
                    hrel_sb[:, mt, r, :],
                    aT[r][:, mt, c * 128:(c + 1) * 128],
                    start=False, stop=False)
                m1_count[c] += 1
                emitted.add((r, mt, c))

            def flush_b1():
                for rho in sorted(prepped):
                    for c8 in range(P):
                        mt = c8 * NB + rho
                        for r in range(R):
                            for c in range(NB):
                                if ((r, mt, c) not in emitted
                                        and (c, r) in chunk_done):
                                    emit_b1_single(r, mt, c)

            def emit_prep(rho):
                # hrel1[mt] = (h0^T chunk)^T-stationary @ w1r, natural out
                for c8 in range(P):
                    mt = c8 * NB + rho
                    ps = psHrp.tile([128, R, D], F32, tag="hr", name="hr")
                    for r in range(R):
                        nc.tensor.matmul(ps[:, r, :],
                                         h0Tb[rho][:, c8, :],
                                         w1r_sb[:, r, :],
                                         start=True, stop=True)
                    nc.vector.tensor_copy(hrel_sb[:, mt, :, :], ps[:])
                prepped.add(rho)

            # ---- main load loop: block-major, relation-inner ----
            with (
                tc.tile_pool(name="nat16", bufs=3) as natp,
                tc.tile_pool(name="pst", bufs=2, space="PSUM") as pstp,
                tc.tile_pool(name="psB0", bufs=2, space="PSUM") as psB0p,
                tc.tile_pool(name="psM0", bufs=1, space="PSUM") as psM0p,
                tc.tile_pool(name="b0sb", bufs=2) as b0p,
            ):
                open_group(psM1[:], NP)
                for b in range(NB):
                    psM0 = psM0p.tile([D, 128], F32, tag="m0", name="psM0")
                    # self term opens the msg0 accumulation group
                    nc.tensor.matmul(psM0[:], w0s_sb[:],
                                     x0To_sb[:, b * 128:(b + 1) * 128],
                                     start=True, stop=False)
                    ps_B0 = psB0p.tile([2 * T, R, 128], F32, tag="b0",
                                       name="ps_B0")
                    open_group(ps_B0[:], R * 128)
                    for r in range(R):
                        # chunk (b, r): 2 half-stages [128, 2048] (8KB/row
                        # descriptors keep the SWDGE at the HBM bus rate);
                        # the very first and last halves split in two so
                        # the PE pipeline starts early / drains early.
                        for half in range(2):
                            nat = natp.tile([128, N // 2], F16, tag="nat",
                                            name="nat16")
                            m0 = half * 2048
                            first = b == 0 and r == 0 and half == 0
                            last = (b == NB - 1 and r == R - 1
                                    and half == 1)
                            if first or last:
                                for sub in range(2):
                                    nc.gpsimd.dma_start(
                                        nat[:, sub * 1024:(sub + 1) * 1024],
                                        a_sh.ap()[r, b * 128:(b + 1) * 128,
                                                  m0 + sub * 1024:
                                                  m0 + (sub + 1) * 1024])
                            else:
                                nc.gpsimd.dma_start(
                                    nat[:],
                                    a_sh.ap()[r, b * 128:(b + 1) * 128,
                                              m0:m0 + 2048])
                            if r == 0 and half == 0 and pending_cc:
                                for fn in pending_cc:
                                    fn()
                                pending_cc = []
                            for q in range(2):
                                pst = pstp.tile([128, 8, 128], F16,
                                                tag="pst", name="pst")
                                for j in range(8):
                                    ml = q * 8 + j
                                    nc.tensor.transpose(
                                        pst[:, j, :],
                                        nat[:, ml * 128:(ml + 1) * 128],
                                        ident_sb[:])
                                mt0 = half * 16 + q * 8
                                dst = aT[r][:, mt0:mt0 + 8,
                                            b * 128:(b + 1) * 128]
                                if (half + q) % 2 == 0:
                                    nc.vector.tensor_copy(dst, pst[:])
                                else:
                                    nc.scalar.copy(dst, pst[:])
                                for j in range(8):
                                    mt = mt0 + j
                                    nc.tensor.matmul(
                                        ps_B0[:, r, :],
                                        x0n_sb[:, mt, :],
                                        aT[r][:, mt,
                                              b * 128:(b + 1) * 128],
                                        start=False,
                                        stop=(r == R - 1 and mt == MT - 1))
                        chunk_done.add((b, r))
                        if b == NB - 1 and r == R - 1:
                            break  # keep the h0ch3 chain tight; flush later
                        for rho, (pb, pr) in PREP_SLOT.items():
                            if (pb, pr) == (b, r):
                                emit_prep(rho)
                        flush_b1()

                    # ---- end of block b: msg0 chunk -> h0 chunk -> AG ----
                    B0sb = b0p.tile([2 * T, R, 128], F32, tag="b0s",
                                    name="B0sb")
                    nc.vector.tensor_copy(B0sb[:], ps_B0[:])
                    for r in range(R):
                        nc.tensor.matmul(psM0[:], w0r_sb[:, r, :],
                                         B0sb[:, r, :],
                                         start=False, stop=(r == R - 1))
                    nc.scalar.activation(h0ch[b][:], psM0[:], AF.Tanh,
                                         bias=bias(0))
                    nc.sync.dma_start(ag_in[b][:], h0ch[b][:])

                    def emit_ag(b=b):
                        nc.gpsimd.collective_compute(
                            "AllGather", mybir.AluOpType.bypass,
                            replica_groups=[list(range(P))],
                            ins=[ag_in[b][:]], outs=[ag_out[b][:]])
                        nc.sync.dma_start(
                            h0Tb[b][:],
                            ag_out[b][:].rearrange("c d n -> d c n"))

                    if b == NB - 1:
                        emit_ag()  # no next block; the load is done anyway
                    else:
                        pending_cc.append(emit_ag)
                    # w1s self term joins the psM1 column group b
                    nc.tensor.matmul(psM1[:, b * 128:(b + 1) * 128],
                                     w1s_sb[:], h0ch[b][:],
                                     start=False, stop=False)
                    m1_count[b] += 1
                    if b == NB - 1:
                        flush_b1()  # drain singles during the AG3 window

            # ---- tail ----
            with (
                tc.tile_pool(name="work", bufs=1) as wp,
                tc.tile_pool(name="psMlp", bufs=2, space="PSUM") as psMlp,
                tc.tile_pool(name="psAux", bufs=2, space="PSUM") as psAux,
            ):
                # x0 halves of the gated MLPs + PE warmup during AG3
                ps_wm = psAux.tile([D, 128], F32, tag="aux", name="ps_wm")
                for _ in range(8):
                    nc.tensor.matmul(ps_wm[:], ident_sb[:], h0ch[0][:],
                                     start=True, stop=True)
                ps_ti = psMlp.tile([D, NP], F32, tag="mlp", name="ps_ti")
                nc.tensor.matmul(ps_ti[:], iw1a_sb[:], x0To_sb[:],
                                 start=True, stop=False)
                ps_tj = psMlp.tile([D, NP], F32, tag="mlp", name="ps_tj")
                nc.tensor.matmul(ps_tj[:], jw1a_sb[:], x0To_sb[:],
                                 start=True, stop=False)

                # last round of layer 1 (after AG3 lands), full width
                emit_prep(NB - 1)
                combos = [(r, c8 * NB + NB - 1)
                          for c8 in range(P) for r in range(R)]
                for i, (r, mt) in enumerate(combos):
                    nc.tensor.matmul(psM1[:], hrel_sb[:, mt, r, :],
                                     aT[r][:, mt, :],
                                     start=False, stop=(i == len(combos) - 1))
                    for c in range(NB):
                        m1_count[c] += 1
                assert all(k == 129 for k in m1_count), m1_count

                h1To = wp.tile([D, NP], F16)
                nc.scalar.activation(h1To[:], psM1[:], AF.Tanh, bias=bias(1))

                # gated i/j MLPs (transposed layout [D, rows])
                nc.tensor.matmul(ps_ti[:], iw1b_sb[:], h1To[:],
                                 start=False, stop=True)
                t_i = wp.tile([D, NP], F16)
                nc.scalar.activation(t_i[:], ps_ti[:], AF.Tanh, bias=bias(2))
                nc.tensor.matmul(ps_tj[:], jw1b_sb[:], h1To[:],
                                 start=False, stop=True)
                t_j = wp.tile([D, NP], F16)
                nc.scalar.activation(t_j[:], ps_tj[:], AF.Tanh, bias=bias(4))

                ps_yi = psMlp.tile([D, NP], F32, tag="mlp", name="ps_yi")
                nc.tensor.matmul(ps_yi[:], iw2_sb[:], t_i[:], start=True,
                                 stop=True)
                i_sb = wp.tile([D, NP], F16)
                nc.scalar.activation(i_sb[:], ps_yi[:], AF.Sigmoid,
                                     bias=bias(3))
                ps_yj = psMlp.tile([D, NP], F32, tag="mlp", name="ps_yj")
                nc.tensor.matmul(ps_yj[:], jw2_sb[:], t_j[:], start=True,
                                 stop=True)
                j_sb = wp.tile([D, NP], F16)
                nc.scalar.activation(j_sb[:], ps_yj[:], AF.Tanh,
                                     bias=bias(5))

                gT = wp.tile([D, NP], F16)
                nc.vector.tensor_mul(gT[:], i_sb[:], j_sb[:])
                ps_g = psAux.tile([128, NB, D], F16, tag="aux", name="ps_g")
                for nt in range(NB):
                    nc.tensor.transpose(ps_g[:, nt, :],
                                        gT[:, nt * 128:(nt + 1) * 128],
                                        ident_sb[:])
                g_nat = wp.tile([128, NB, D], F16)
                nc.vector.tensor_copy(g_nat[:], ps_g[:])

                # segment-sum partial pooled^T [D, G]
                ps_pool = psMlp.tile([D, G], F32, tag="mlp", name="ps_pool")
                for nt in range(NB):
                    nc.tensor.matmul(
                        ps_pool[:], g_nat[:, nt, :], S_sb[:, nt, :],
                        start=(nt == 0), stop=(nt == NB - 1))
                pool_sb = wp.tile([D, G], F16)
                nc.vector.tensor_copy(pool_sb[:], ps_pool[:])
                # to graph-major [G, D] for the ReduceScatter
                ps_pn = psAux.tile([128, NB, D], F16, tag="aux", name="ps_pn")
                for gt in range(NB):
                    nc.tensor.transpose(ps_pn[:, gt, :],
                                        pool_sb[:, gt * 128:(gt + 1) * 128],
                                        ident_sb[:])
                pool_nat = wp.tile([128, NB, D], F16)
                nc.vector.tensor_copy(pool_nat[:], ps_pn[:])
                nc.sync.dma_start(
                    rs_in[:].rearrange("(gt p) d -> p gt d", p=128),
                    pool_nat[:])
                nc.gpsimd.collective_compute(
                    "ReduceScatter", mybir.AluOpType.add,
                    replica_groups=[list(range(P))],
                    ins=[rs_in[:]], outs=[rs_out[:]])
                rs_sb = wp.tile([GC, D], F16)
                nc.sync.dma_start(rs_sb[:], rs_out[:])

                # final MLP on this core's 64-graph slice
                ps_pt = psAux.tile([D, GC], F16, tag="aux", name="ps_pt")
                nc.tensor.transpose(ps_pt[:], rs_sb[:],
                                    ident_sb[0:GC, 0:GC])
                pooled_t = wp.tile([D, GC], F16)
                nc.scalar.activation(pooled_t[:], ps_pt[:], AF.Tanh)
                ps_z = psMlp.tile([D, GC], F32, tag="mlp", name="ps_z")
                nc.tensor.matmul(ps_z[:], fw1_sb[:], pooled_t[:], start=True,
                                 stop=True)
                z1_sb = wp.tile([D, GC], F16)
                nc.scalar.activation(z1_sb[:], ps_z[:], AF.Tanh,
                                     bias=bias(6))
                ps_o = psAux.tile([1, GC], F32, tag="aux", name="ps_o")
                nc.tensor.matmul(ps_o[:], fw2_sb[:], z1_sb[:], start=True,
                                 stop=True)
                out_sb = wp.tile([1, GC], F32)
                nc.scalar.activation(out_sb[:], ps_o[:], AF.Identity,
                                     bias=fb2_sb[:, 0:1])
                nc.sync.dma_start(outT.ap(), out_sb[:])
                if DEBUG_TAPS:
                    for b in range(NB):
                        nc.scalar.dma_start(
                            dbg_h0.ap()[b],
                            h0Tb[b][:].rearrange("d c n -> d (c n)"))
                    nc.scalar.dma_start(dbg_h1.ap(), h1To[:])
                    nc.scalar.dma_start(dbg_w1r.ap(), w1r_sb[:])
                    nc.scalar.dma_start(dbg_hrel.ap(), hrel_sb[:])
                    nc.scalar.dma_start(dbg_pool.ap(), pool_sb[:])
                    nc.scalar.dma_start(dbg_rs.ap(), rs_sb[:])

    nc.compile()
    return nc


def _prep_shared(x0, w0s, w0r, b0, w1s, w1r, b1, iw1, ib1, iw2, ib2,
                 jw1, jb1, jw2, jb2, fw1, fb1, fw2, fb2):
    f16 = np.float16
    f32 = np.float32
    x016 = x0.astype(f16)
    x0lo = (x0 - x016.astype(f32)).astype(f16)
    x0hl = np.concatenate([x016, x0lo], axis=1)  # [N, 2T]
    w0r2 = np.concatenate([w0r, w0r], axis=1)    # [R, 2T, D]
    shared = {
        "x0nat": np.ascontiguousarray(
            x0hl.reshape(MT, 128, 2 * T).transpose(1, 0, 2)),
        "w0r": np.ascontiguousarray(w0r2).astype(f32),
        "w1r": np.ascontiguousarray(w1r).astype(f16),
        "w0s": np.ascontiguousarray(w0s).astype(f32),
        "w1s": np.ascontiguousarray(w1s).astype(f16),
        "iw1a": np.ascontiguousarray(iw1[:T]).astype(f32),
        "iw1b": np.ascontiguousarray(iw1[T:]).astype(f16),
        "iw2": np.ascontiguousarray(iw2).astype(f16),
        "jw1a": np.ascontiguousarray(jw1[:T]).astype(f32),
        "jw1b": np.ascontiguousarray(jw1[T:]).astype(f16),
        "jw2": np.ascontiguousarray(jw2).astype(f16),
        "fw1": np.ascontiguousarray(fw1).astype(f16),
        "fw2": np.ascontiguousarray(fw2).astype(f16),
        "bias8": np.stack(
            [b0, b1, ib1, ib2, jb1, jb2, fb1, np.zeros(D, f32)],
            axis=1).astype(f32),
        "fb2v": np.asarray(fb2, f32).reshape(1, 1),
        "ident": np.eye(128, dtype=f16),
    }
    return shared


def kernel(x0, a, segment_ids,
           w0s, w0r, b0, w1s, w1r, b1,
           iw1, ib1, iw2, ib2,
           jw1, jb1, jw2, jb2,
           fw1, fb1, fw2, fb2):
    if "nc" not in _CACHE:
        _CACHE["nc"] = _build()
    nc = _CACHE["nc"]

    x0 = np.asarray(x0, np.float32)
    a = np.asarray(a, np.float32)
    segment_ids = np.asarray(segment_ids)

    shared = _prep_shared(x0, w0s, w0r, b0, w1s, w1r, b1, iw1, ib1, iw2,
                          ib2, jw1, jb1, jw2, jb2, fw1, fb1, fw2, fb2)
    x0T32 = x0.T.astype(np.float32)
    gids = np.arange(G, dtype=segment_ids.dtype)
    in_maps = []
    for c in range(P):
        sl = slice(c * NP, (c + 1) * NP)
        m = dict(shared)
        m["a_sh"] = np.ascontiguousarray(a[:, sl, :])
        m["x0To"] = np.ascontiguousarray(x0T32[:, sl])
        m["Sm"] = (segment_ids[sl, None] == gids[None, :]).astype(np.float16)
        in_maps.append(m)

    res = bass_utils.run_bass_kernel_spmd(nc, in_maps,
                                          core_ids=list(range(P)))
    out = np.concatenate(
        [np.asarray(res.results[c]["outT"], np.float32).reshape(GC)
         for c in range(P)])
    return out.reshape(G, 1)

